# revision 1
# baseline (speedup 1.0000x reference)
"""Multi-head cross-attention kernel for Trainium2, 8 NeuronCores.

Reference computation (B=2, S=2048, D=1024, H=16, hd=64):
    kv = x @ Wkv + bkv ; q = y @ Wq + bq
    per head: s = q k^T / 8 (+ mask, all-zero per spec), a = softmax(s)
    out = concat_h(a v) @ Wo + bo

Sharding: batch (2-way) x head-groups (4 heads/core), fully collective-free.
Core c owns batch c//4 and heads 4j..4j+3 (j = c%4).  Each core computes a
PARTIAL output projection out_c = softmax(qk)v @ Wo[256-row slice] + bo/4
over the full S of its batch; the host sums the 4 partials per batch.  This
replaces the previous design's two AllToAlls (43+23 us at 10-23 GB/s bus
bandwidth) with 8.4 MB of fully-overlapped output DMA.

The kernel is engine-balance driven (all matmuls fp16, fp32 PSUM):
  - ACT owns exp: 128 N=1024 ACTIVATEs ~= 147 us of irreducible work.
  - PE owns ~175 us of streaming at the observed ~2 GHz (GPIO-throttled)
    clock: projections, row-packed concurrent K=64 score pairs (two heads
    per 2-bank PSUM tile at tile_position (0,0)/(64,0) measured starting
    4 ns apart), M=65 PV matmuls whose extra ones-column accumulates the
    softmax denominator, and the partial outproj.
  - Everything else hides under those two: input DMA is consolidated into
    single dma_starts per tensor/slice (a dma_start costs ~1 us setup);
    kT/v/q projection slices and outproj units are emitted inside the
    attention chunk loop to fill PE slack; each pair epilogue (DVE
    reciprocal of the denominator row, ones-matmul broadcast into the
    just-freed PV bank, DVE normalize into SBUF fp16) gets a full pair
    window to complete by alternating PV accumulators between two PSUM
    pools (even pairs pvA, odd pairs pvB).

PSUM budget (8 banks): scores 2x[128,1024] double-buffer (4) + pvA (2) +
pvB (2); projections, rep broadcasts and outproj units recycle whichever
pv pool is idle in their window.
"""

import numpy as np

import concourse.bass as bass
import concourse.bacc as bacc
import concourse.mybir as mybir
from concourse.tile import TileContext
from concourse.bass_utils import run_bass_kernel_spmd

B, S, D = 2, 2048, 1024
H, HD = 16, 64
N_CORES = 8
GROUP = 4              # cores per batch group
HPC = H // GROUP       # heads per core (4)
NV = HPC * HD          # local vals rows (256)
SQB = 512              # sq block size
NBLK = S // SQB        # 4
NKC = S // 128         # 16 sk chunks
NDC = D // 128         # 8 contraction chunks
SKB = 512              # sk/sq slice size for projections

F32 = mybir.dt.float32
FP16 = mybir.dt.float16
EXP = mybir.ActivationFunctionType.Exp
LOG = mybir.ActivationFunctionType.Ln


def build_kernel():
    nc = bacc.Bacc("TRN2", target_bir_lowering=False, debug=False,
                   num_devices=N_CORES)

    yT = nc.declare_dram_parameter("yT", [D, S], FP16, isOutput=False)
    xT = nc.declare_dram_parameter("xT", [D, S], FP16, isOutput=False)
    wq = nc.declare_dram_parameter("wq", [D, NV], FP16, isOutput=False)
    wk = nc.declare_dram_parameter("wk", [D, NV], FP16, isOutput=False)
    wv = nc.declare_dram_parameter("wv", [D, NV], FP16, isOutput=False)
    wo = nc.declare_dram_parameter("wo", [NV, D], FP16, isOutput=False)
    bq = nc.declare_dram_parameter("bq", [NV], F32, isOutput=False)
    bo = nc.declare_dram_parameter("bo", [D], F32, isOutput=False)
    outp = nc.declare_dram_parameter("outp", [S, D], F32, isOutput=True)

    inv_sqrt_hd = float(1.0 / np.sqrt(HD))

    with TileContext(nc) as tc:
        with (
            tc.tile_pool(name="acts", bufs=1) as acts,        # persistent
            tc.tile_pool(name="wts", bufs=1) as wts,
            tc.tile_pool(name="xys", bufs=2) as xys,          # proj streaming
            tc.tile_pool(name="stream", bufs=3) as stream,
            tc.tile_pool(name="attn", bufs=4) as attn,        # exp(scores)
            tc.tile_pool(name="psc", bufs=2, space="PSUM") as psc,
            tc.tile_pool(name="pva", bufs=2, space="PSUM") as pva,
            tc.tile_pool(name="pvb", bufs=2, space="PSUM") as pvb,
        ):
            # ---- persistent tiles ----
            qT_sb = [acts.tile([128, S], FP16, tag=f"qT{i}", name=f"qT{i}")
                     for i in range(2)]
            kT_sb = [acts.tile([128, S], FP16, tag=f"kT{i}", name=f"kT{i}")
                     for i in range(2)]
            v_sb = [acts.tile([128, HPC * (HD + 1)], FP16, tag=f"v{i}",
                              name=f"v{i}") for i in range(NKC)]
            nv_sb = [acts.tile([128, S], FP16, tag=f"nv{i}", name=f"nv{i}")
                     for i in range(2)]
            ones65 = acts.tile([65, 128], FP16, tag="ones65")
            dstage = acts.tile([65, 2 * SQB], F32, tag="dstage")
            dlog = acts.tile([65, 2 * SQB], F32, tag="dlog")
            drec_h = acts.tile([65, 2 * SQB], FP16, tag="drec_h")
            bq_sb = acts.tile([128, 2], F32, tag="bq")
            bo_bc = acts.tile([128, D], F32, tag="bo_bc")
            warm = acts.tile([1, 8], F32, tag="warm")

            nc.vector.memset(ones65[:], 1.0)
            # preload the exp table set while the input DMA streams
            nc.vector.memset(warm[:], 0.0)
            nc.scalar.activation(warm[:], warm[:], EXP)

            # weights, one dma_start per tensor: [D, M] -> [128, NDC*M]
            # with contraction-chunk-major columns.  wq first (first
            # projection consumes it); bo/wo deferred past the preamble.
            wk_sb = wts.tile([128, NDC * NV], FP16, tag="wk")
            wv_sb = wts.tile([128, NDC * NV], FP16, tag="wv")
            wq_sb = wts.tile([128, NDC * NV], FP16, tag="wq")
            wo_sb = wts.tile([128, 2 * D], FP16, tag="wo")
            nc.sync.dma_start(out=bq_sb[:],
                              in_=bq.rearrange("(c p) -> p c", p=128))
            nc.sync.dma_start(
                out=wq_sb[:].rearrange("p (c m) -> p c m", c=NDC),
                in_=wq.rearrange("(c p) m -> p c m", p=128))
            nc.sync.dma_start(
                out=wk_sb[:].rearrange("p (c m) -> p c m", c=NDC),
                in_=wk.rearrange("(c p) m -> p c m", p=128))
            nc.sync.dma_start(
                out=wv_sb[:].rearrange("p (c m) -> p c m", c=NDC),
                in_=wv.rearrange("(c p) m -> p c m", p=128))

            def load_tail_params():
                nc.sync.dma_start(
                    out=bo_bc[:], in_=bo[None, :].to_broadcast((128, D)))
                nc.sync.dma_start(
                    out=wo_sb[:].rearrange("p (c m) -> p c m", c=2),
                    in_=wo.rearrange("(c p) m -> p c m", p=128))

            # ---- emission helpers ----
            def load_slice(src, sb):
                t = xys.tile([128, NDC * SKB], FP16, tag="xys", name="xys")
                nc.sync.dma_start(
                    out=t[:].rearrange("p (c m) -> p c m", c=NDC),
                    in_=src[:, SKB * sb:SKB * (sb + 1)]
                    .rearrange("(c p) m -> p c m", p=128))
                return t

            def proj_kv(sb, pool, tag):
                xt = load_slice(xT, sb)
                for cc in range(2):
                    ps = pool.tile([128, SQB], F32, tag=tag, name="ps_k")
                    for i in range(NDC):
                        nc.tensor.matmul(
                            ps[:],
                            wk_sb[:, NV * i + 128 * cc:
                                  NV * i + 128 * (cc + 1)],
                            xt[:, SKB * i:SKB * (i + 1)],
                            start=(i == 0), stop=(i == NDC - 1))
                    nc.vector.tensor_copy(
                        kT_sb[cc][:, SKB * sb:SKB * (sb + 1)], ps[:])
                for sc4 in range(SKB // 128):
                    ps = pool.tile([128, SQB], F32, tag=tag, name="ps_v")
                    ps = ps[:, :NV]
                    for i in range(NDC):
                        nc.tensor.matmul(
                            ps[:],
                            xt[:, SKB * i + 128 * sc4:
                               SKB * i + 128 * (sc4 + 1)],
                            wv_sb[:, NV * i:NV * (i + 1)],
                            start=(i == 0), stop=(i == NDC - 1))
                    ks = sb * (SKB // 128) + sc4
                    nc.vector.memset(v_sb[ks][:], 1.0)
                    nc.vector.tensor_copy(
                        v_sb[ks][:].rearrange("p (h c) -> p h c",
                                              c=HD + 1)[:, :, 0:HD],
                        ps[:].rearrange("p (h c) -> p h c", c=HD))

            def proj_q(sb, pool, tag):
                yt = load_slice(yT, sb)
                for cc in range(2):
                    ps = pool.tile([128, SQB], F32, tag=tag, name="ps_q")
                    for i in range(NDC):
                        nc.tensor.matmul(
                            ps[:],
                            wq_sb[:, NV * i + 128 * cc:
                                  NV * i + 128 * (cc + 1)],
                            yt[:, SKB * i:SKB * (i + 1)],
                            start=(i == 0), stop=(i == NDC - 1))
                    nc.vector.tensor_scalar_add(
                        qT_sb[cc][:, SKB * sb:SKB * (sb + 1)], ps[:],
                        bq_sb[:, cc:cc + 1])

            def attn_scores(blk, pair, sc):
                """Row-packed concurrent score pair + one N=1024 exp."""
                sq0 = SQB * blk
                sc_ps = psc.tile([128, 2 * SQB], F32, tag="sc", name="sc_ps")
                at = attn.tile([128, 2 * SQB], FP16, tag="at", name="at")
                for hh in range(2):
                    nc.tensor.matmul(
                        sc_ps[:, SQB * hh:SQB * (hh + 1)],
                        kT_sb[pair][64 * hh:64 * (hh + 1),
                                    128 * sc:128 * (sc + 1)],
                        qT_sb[pair][64 * hh:64 * (hh + 1), sq0:sq0 + SQB],
                        tile_position=(64 * hh, 0))
                nc.scalar.activation(at[:], sc_ps[:], EXP, scale=inv_sqrt_hd)
                return at

            def attn_pv(pair, sc, at, pv_ps):
                for hh in range(2):
                    h = 2 * pair + hh
                    nc.tensor.matmul(
                        pv_ps[hh][:],
                        v_sb[sc][:, (HD + 1) * h:(HD + 1) * (h + 1)],
                        at[:, SQB * hh:SQB * (hh + 1)],
                        start=(sc == 0), stop=(sc == NKC - 1))

            def epilogue_a(blk, pair, pv_ps):
                """Drain the PV banks with cheap copies FIRST (the banks
                are a shared arena -- the next pair's PV matmuls wait on
                their release, so nothing slow may precede these in the
                DVE queue), then run the ~3 us/row iterative reciprocals
                from the SBUF staging rows."""
                sq0 = SQB * blk
                for hh in range(2):
                    col = slice(SQB * hh, SQB * (hh + 1))
                    nc.vector.tensor_copy(
                        nv_sb[pair][64 * hh:64 * (hh + 1), sq0:sq0 + SQB],
                        pv_ps[hh][0:HD, :])
                    nc.vector.tensor_copy(dstage[64:65, col],
                                          pv_ps[hh][HD:HD + 1, :])
                # quartered reciprocals: DVE reciprocal cost scales
                # with free size, so the first [1,128] quarter (and its
                # fp16 cast, the rep matmul's dependency) is ready ~3 us
                # after the pair ends instead of ~6
                for hh in range(2):
                    for qq in range(4):
                        cs = slice(SQB * hh + 128 * qq,
                                   SQB * hh + 128 * (qq + 1))
                        nc.vector.reciprocal(dlog[64:65, cs],
                                             dstage[64:65, cs])
                        nc.vector.tensor_copy(drec_h[64:65, cs],
                                              dlog[64:65, cs])

            def epilogue_b(blk, pair, pool, tag, hh):
                """Broadcast 1/d over 64 partitions via a ones matmul into
                a freed PV bank, then normalize in place.  Fired late in
                the next pair (sc 12/15) so the ~9 us DVE reciprocal chain
                is done before the PE FIFO reaches the matmul."""
                sq0 = SQB * blk
                nv_sl = nv_sb[pair][64 * hh:64 * (hh + 1), sq0:sq0 + SQB]
                col = slice(SQB * hh, SQB * (hh + 1))
                rep = pool.tile([128, SQB], F32, tag=tag, name="rep")
                for qq in range(4):
                    cs = slice(SQB * hh + 128 * qq,
                               SQB * hh + 128 * (qq + 1))
                    nc.tensor.matmul(rep[:, 128 * qq:128 * (qq + 1)],
                                     ones65[64:65, :], drec_h[64:65, cs],
                                     tile_position=(64, 0))
                nc.vector.tensor_mul(nv_sl, nv_sl, rep[0:HD, :])

            osb_box = [None]

            def outproj_unit(blk, m, dcb, pool, tag):
                """One (sq 128-chunk, 512-col) slice of the partial output
                projection, bias added on DVE eviction."""
                sq0 = SQB * blk
                if dcb == 0:
                    osb_box[0] = stream.tile([128, D], F32, tag="o_sb",
                                             name="o_sb")
                o_sb = osb_box[0]
                o_ps = pool.tile([128, SQB], F32, tag=tag, name="o_ps")
                for pair in range(2):
                    nc.tensor.matmul(
                        o_ps[:],
                        nv_sb[pair][:, sq0 + 128 * m:sq0 + 128 * (m + 1)],
                        wo_sb[:, D * pair + 512 * dcb:
                              D * pair + 512 * (dcb + 1)],
                        start=(pair == 0), stop=(pair == 1))
                nc.vector.tensor_add(o_sb[:, 512 * dcb:512 * (dcb + 1)],
                                     o_ps[:], bo_bc[:, 512 * dcb:
                                                    512 * (dcb + 1)])
                if dcb == 1:
                    nc.sync.dma_start(
                        out=outp[sq0 + 128 * m:sq0 + 128 * (m + 1), :],
                        in_=o_sb[:])

            # ---- emission schedule ----
            # preamble: first projection slices (ACT idle anyway)
            proj_q(0, pvb, "pvB")
            proj_kv(0, pvb, "pvB")
            load_tail_params()

            prev_pv = None     # (blk, pair, tiles, pool, tag) pending epi
            prev_blk_done = -1  # last blk whose outproj has been emitted
            for blk in range(NBLK):
                for pair in range(2):
                    pool, tag = (pva, "pvA") if pair == 0 else (pvb, "pvB")
                    pv_ps = [pool.tile([128, SQB], F32, tag=tag,
                                       name=f"pv{hh}")[:HD + 1, :]
                             for hh in range(2)]
                    # keep ACT busy across the transition: two chunks of
                    # scores+exp first, then drain the previous pair
                    ats = [attn_scores(blk, pair, 0),
                           attn_scores(blk, pair, 1)]
                    pending_epi_b = None
                    if prev_pv is not None:
                        pblk, ppair, ptiles, ppool, ptag = prev_pv
                        epilogue_a(pblk, ppair, ptiles)
                        pending_epi_b = (pblk, ppair, ppool, ptag)
                    attn_pv(pair, 0, ats[0], pv_ps)
                    attn_pv(pair, 1, ats[1], pv_ps)
                    # interleaved fill work for the PE in this window;
                    # {fire_after_chunk: [emissions]}.  kT/v slice sb MUST
                    # be emitted before chunk 4*sb reads it; outproj of
                    # blk-1 runs in blk's pair1 window, after blk-1 pair1's
                    # normalize muls (fired at sc 9/12 of pair0, when the
                    # DVE reciprocal chain is guaranteed done).
                    fills = {}
                    if blk == 0 and pair == 0:
                        fills = {3: [lambda: proj_kv(1, pvb, "pvB")],
                                 7: [lambda: proj_kv(2, pvb, "pvB")],
                                 11: [lambda: proj_kv(3, pvb, "pvB")]}
                    elif blk == 0 and pair == 1:
                        fills = {4: [lambda: proj_q(1, pva, "pvA")],
                                 8: [lambda: proj_q(2, pva, "pvA")],
                                 12: [lambda: proj_q(3, pva, "pvA")]}
                    elif pair == 1:
                        # late (sc 8-15): the first unit's bank recycle
                        # waits on DVE bias-adds queued behind the pair0
                        # epilogue reciprocals; firing earlier stalls the
                        # PE FIFO ~4 us at every block transition
                        fills = {s + 8: [lambda u=u: outproj_unit(
                            blk - 1, u // 2, u % 2, pva, "pvA")]
                            for u in range(2 * (SQB // 128))
                            for s in (u,)}
                    for sc in range(2, NKC):
                        at = attn_scores(blk, pair, sc)
                        attn_pv(pair, sc, at, pv_ps)
                        if pending_epi_b is not None and sc in (12, 15):
                            epilogue_b(*pending_epi_b, hh=(0 if sc == 12
                                                           else 1))
                        for f in fills.get(sc, ()):
                            f()
                    prev_pv = (blk, pair, pv_ps, pool, tag)

            # tail: last pair epilogue + last block outproj
            pblk, ppair, ptiles, ppool, ptag = prev_pv
            epilogue_a(pblk, ppair, ptiles)
            epilogue_b(pblk, ppair, ppool, ptag, hh=0)
            epilogue_b(pblk, ppair, ppool, ptag, hh=1)
            for u in range(2 * (SQB // 128)):
                outproj_unit(NBLK - 1, u // 2, u % 2, pva, "pvA")

    nc.compile()
    return nc


last_results = None


def kernel(x, y, mask, Wkv, bkv, Wq, bq, Wo, bo):
    x = np.asarray(x, dtype=np.float32)
    y = np.asarray(y, dtype=np.float32)
    Wkv = np.asarray(Wkv, dtype=np.float32)
    bkv = np.asarray(bkv, dtype=np.float32)
    Wq = np.asarray(Wq, dtype=np.float32)
    bq = np.asarray(bq, dtype=np.float32)
    Wo = np.asarray(Wo, dtype=np.float32)
    bo = np.asarray(bo, dtype=np.float32)

    wkv3 = Wkv.reshape(D, H, 2 * HD)
    bv = bkv.reshape(H, 2 * HD)[:, HD:].reshape(H * HD)
    # v-bias folded into the output bias; each of the 4 partial sums per
    # batch carries bo_eff/4 so the host-side reduce reproduces bo_eff.
    bo_eff4 = ((bv @ Wo + bo) / GROUP).astype(np.float32)

    nc = build_kernel()
    in_maps = []
    for c in range(N_CORES):
        b, j = divmod(c, GROUP)
        hs = HPC * j
        f16 = np.float16
        in_maps.append({
            "yT": np.ascontiguousarray(y[b].T).astype(f16),
            "xT": np.ascontiguousarray(x[b].T).astype(f16),
            "wq": np.ascontiguousarray(
                Wq[:, HD * hs:HD * (hs + HPC)]).astype(f16),
            "wk": np.ascontiguousarray(
                wkv3[:, hs:hs + HPC, :HD].reshape(D, NV)).astype(f16),
            "wv": np.ascontiguousarray(
                wkv3[:, hs:hs + HPC, HD:].reshape(D, NV)).astype(f16),
            "wo": np.ascontiguousarray(
                Wo[HD * hs:HD * (hs + HPC), :]).astype(f16),
            "bq": np.ascontiguousarray(bq[HD * hs:HD * (hs + HPC)]),
            "bo": bo_eff4,
        })

    import os
    trace = bool(os.environ.get("KERNEL_TRACE"))
    res = run_bass_kernel_spmd(nc, in_maps, core_ids=list(range(N_CORES)),
                               trace=trace)
    global last_results
    last_results = res

    full = np.empty((B, S, D), dtype=np.float32)
    for b in range(B):
        acc = res.results[GROUP * b]["outp"].astype(np.float32)
        for j in range(1, GROUP):
            acc = acc + res.results[GROUP * b + j]["outp"]
        full[b] = acc
    return full



# revision 5
# speedup vs baseline: 1.0259x; 1.0259x over previous
"""Multi-head cross-attention kernel for Trainium2, 8 NeuronCores.

Reference computation (B=2, S=2048, D=1024, H=16, hd=64):
    kv = x @ Wkv + bkv ; q = y @ Wq + bq
    per head: s = q k^T / 8 (+ mask, all-zero per spec), a = softmax(s)
    out = concat_h(a v) @ Wo + bo

Sharding: batch (2-way) x head-groups (4 heads/core), fully collective-free.
Core c owns batch c//4 and heads 4j..4j+3 (j = c%4).  Each core computes a
PARTIAL output projection out_c = softmax(qk)v @ Wo[256-row slice] + bo/4
over the full S of its batch; the host sums the 4 partials per batch (fp16
partials; quantization ~5e-4 of partial scale, well under the 2e-2 budget).

The kernel is engine-balance driven (all matmuls fp16, fp32 PSUM):
  - ACT owns exp: 128 N=1024 ACTIVATEs ~= 147 us of irreducible work and
    paces the steady-state chunk loop at ~1.15 us/chunk.
  - PE owns the matmul stream at the observed ~2 GHz (GPIO-throttled)
    clock: projections, row-packed concurrent K=64 score pairs (two heads
    per 2-bank PSUM tile at tile_position (0,0)/(64,0)), M=65 PV matmuls
    whose extra ones-column accumulates the softmax denominator, and the
    partial outproj.  A dummy-matmul spin on memset data covers the input
    DMA preamble so HAM un-throttles before the first real matmul.
  - DVE drains PSUM and normalizes.  The softmax reciprocal uses the
    single-pass reciprocal_approx_fast custom op (~51 ULP, ~5x cheaper
    than the iterative reciprocal) on [1,512] rows.
  - GPSIMD (otherwise idle) broadcasts 1/d from one partition to the 64
    v-dim partitions via partition_broadcast, replacing the old
    ones-matmul rep broadcast on the PE.
  - Input DMA is ordered by first use (wq, y-slice0, wk, x-slice0, wv,
    then the rest); kT/v/q projection slices and outproj units are
    emitted inside the attention chunk loop to fill PE slack.  Outproj
    for blk-1 is split 4+4 units across blk's pair0/pair1 windows to
    balance PE work against the ACT envelope.

PSUM budget (8 banks): scores 2x[128,1024] double-buffer (4) + pvA (2) +
pvB (2); projections and outproj units recycle whichever pv pool had its
accumulators drained at the current window's start.
"""

import numpy as np

import concourse.bass as bass
import concourse.bacc as bacc
import concourse.mybir as mybir
from concourse.tile import TileContext
from concourse.bass_utils import run_bass_kernel_spmd

B, S, D = 2, 2048, 1024
H, HD = 16, 64
N_CORES = 8
GROUP = 4              # cores per batch group
HPC = H // GROUP       # heads per core (4)
NV = HPC * HD          # local vals rows (256)
SQB = 512              # sq block size
NBLK = S // SQB        # 4
NKC = S // 128         # 16 sk chunks
NDC = D // 128         # 8 contraction chunks
SKB = 512              # sk/sq slice size for projections

F32 = mybir.dt.float32
FP16 = mybir.dt.float16
EXP = mybir.ActivationFunctionType.Exp


def build_kernel():
    nc = bacc.Bacc("TRN2", target_bir_lowering=False, debug=False,
                   num_devices=N_CORES)

    yT = nc.declare_dram_parameter("yT", [D, S], FP16, isOutput=False)
    xT = nc.declare_dram_parameter("xT", [D, S], FP16, isOutput=False)
    wq = nc.declare_dram_parameter("wq", [D, NV], FP16, isOutput=False)
    wk = nc.declare_dram_parameter("wk", [D, NV], FP16, isOutput=False)
    wv = nc.declare_dram_parameter("wv", [D, NV], FP16, isOutput=False)
    wo = nc.declare_dram_parameter("wo", [NV, D], FP16, isOutput=False)
    bq = nc.declare_dram_parameter("bq", [NV], F32, isOutput=False)
    bo = nc.declare_dram_parameter("bo", [D], F32, isOutput=False)
    outp = nc.declare_dram_parameter("outp", [S, D], FP16, isOutput=True)

    inv_sqrt_hd = float(1.0 / np.sqrt(HD))

    with TileContext(nc) as tc:
        with (
            tc.tile_pool(name="acts", bufs=1) as acts,        # persistent
            tc.tile_pool(name="wts", bufs=1) as wts,
            tc.tile_pool(name="xys", bufs=2) as xys,          # proj streaming
            tc.tile_pool(name="stream", bufs=3) as stream,
            tc.tile_pool(name="attn", bufs=4) as attn,        # exp(scores)
            tc.tile_pool(name="psc", bufs=2, space="PSUM") as psc,
            tc.tile_pool(name="pva", bufs=2, space="PSUM") as pva,
            tc.tile_pool(name="pvb", bufs=2, space="PSUM") as pvb,
        ):
            # ---- persistent tiles ----
            qT_sb = [acts.tile([128, S], FP16, tag=f"qT{i}", name=f"qT{i}")
                     for i in range(2)]
            kT_sb = [acts.tile([128, S], FP16, tag=f"kT{i}", name=f"kT{i}")
                     for i in range(2)]
            v_sb = [acts.tile([128, HPC * (HD + 1)], FP16, tag=f"v{i}",
                              name=f"v{i}") for i in range(NKC)]
            nv_sb = [acts.tile([128, S], FP16, tag=f"nv{i}", name=f"nv{i}")
                     for i in range(2)]
            # softmax denominator staging: per head, [1,512] rows at
            # partition 0 (copied cross-partition off the PV banks' row 64)
            dstg = [acts.tile([1, SQB], F32, tag=f"dstg{h}", name=f"dstg{h}")
                    for h in range(2)]
            dinv = [acts.tile([1, SQB], F32, tag=f"dinv{h}", name=f"dinv{h}")
                    for h in range(2)]
            dinvh = [acts.tile([1, SQB], FP16, tag=f"dinvh{h}", name=f"dinvh{h}")
                     for h in range(2)]
            rep_sb = [acts.tile([128, SQB], FP16, tag=f"rep{h}",
                                name=f"rep{h}") for h in range(2)]
            bq_sb = acts.tile([128, 2], F32, tag="bq")
            bo_bc = acts.tile([128, D], F32, tag="bo_bc")
            warm = acts.tile([1, 8], F32, tag="warm")
            wones = acts.tile([64, 256], FP16, tag="wones")

            # preload the exp table set + HAM warmup while input DMA streams:
            # ~26 dummy matmuls (~5.5 us cold) keep the PE busy through the
            # weight/slice0 DMA wait so the activity throttle lifts before
            # the first projection matmul.
            nc.vector.memset(warm[:], 0.0)
            nc.scalar.activation(warm[:], warm[:], EXP)
            nc.vector.memset(wones[:], 0.0)
            warm_ps = pva.tile([128, SQB], F32, tag="pvA", name="warm_ps")
            for _ in range(26):
                nc.tensor.matmul(warm_ps[:, 0:256], wones[:, 0:128],
                                 wones[:, 0:256])

            # weights + first slices, one dma_start per tensor, ordered by
            # first use: wq -> y slice0 (q proj), wk -> x slice0 (k proj),
            # wv (v proj).  bo/wo deferred past the preamble.
            wk_sb = wts.tile([128, NDC * NV], FP16, tag="wk")
            wv_sb = wts.tile([128, NDC * NV], FP16, tag="wv")
            wq_sb = wts.tile([128, NDC * NV], FP16, tag="wq")
            wo_sb = wts.tile([128, 2 * D], FP16, tag="wo")
            nc.sync.dma_start(out=bq_sb[:],
                              in_=bq.rearrange("(c p) -> p c", p=128))
            nc.sync.dma_start(
                out=wq_sb[:].rearrange("p (c m) -> p c m", c=NDC),
                in_=wq.rearrange("(c p) m -> p c m", p=128))

            def load_tail_params():
                nc.sync.dma_start(
                    out=bo_bc[:], in_=bo[None, :].to_broadcast((128, D)))
                nc.sync.dma_start(
                    out=wo_sb[:].rearrange("p (c m) -> p c m", c=2),
                    in_=wo.rearrange("(c p) m -> p c m", p=128))

            # ---- emission helpers ----
            def load_slice(src, sb):
                t = xys.tile([128, NDC * SKB], FP16, tag="xys", name="xys")
                nc.sync.dma_start(
                    out=t[:].rearrange("p (c m) -> p c m", c=NDC),
                    in_=src[:, SKB * sb:SKB * (sb + 1)]
                    .rearrange("(c p) m -> p c m", p=128))
                return t

            def load_wk():
                nc.sync.dma_start(
                    out=wk_sb[:].rearrange("p (c m) -> p c m", c=NDC),
                    in_=wk.rearrange("(c p) m -> p c m", p=128))

            def load_wv():
                nc.sync.dma_start(
                    out=wv_sb[:].rearrange("p (c m) -> p c m", c=NDC),
                    in_=wv.rearrange("(c p) m -> p c m", p=128))

            def proj_kv(sb, pool, tag, xt=None):
                if xt is None:
                    xt = load_slice(xT, sb)
                for cc in range(2):
                    ps = pool.tile([128, SQB], F32, tag=tag, name="ps_k")
                    for i in range(NDC):
                        nc.tensor.matmul(
                            ps[:],
                            wk_sb[:, NV * i + 128 * cc:
                                  NV * i + 128 * (cc + 1)],
                            xt[:, SKB * i:SKB * (i + 1)],
                            start=(i == 0), stop=(i == NDC - 1))
                    nc.vector.tensor_copy(
                        kT_sb[cc][:, SKB * sb:SKB * (sb + 1)], ps[:])
                for sc4 in range(SKB // 128):
                    ps = pool.tile([128, SQB], F32, tag=tag, name="ps_v")
                    ps = ps[:, :NV]
                    for i in range(NDC):
                        nc.tensor.matmul(
                            ps[:],
                            xt[:, SKB * i + 128 * sc4:
                               SKB * i + 128 * (sc4 + 1)],
                            wv_sb[:, NV * i:NV * (i + 1)],
                            start=(i == 0), stop=(i == NDC - 1))
                    ks = sb * (SKB // 128) + sc4
                    # ones columns for the in-matmul denominator row
                    nc.vector.memset(
                        v_sb[ks][:].rearrange("p (h c) -> p h c",
                                              c=HD + 1)[:, :, HD:HD + 1],
                        1.0)
                    nc.vector.tensor_copy(
                        v_sb[ks][:].rearrange("p (h c) -> p h c",
                                              c=HD + 1)[:, :, 0:HD],
                        ps[:].rearrange("p (h c) -> p h c", c=HD))

            def proj_q(sb, pool, tag):
                yt = load_slice(yT, sb)
                for cc in range(2):
                    ps = pool.tile([128, SQB], F32, tag=tag, name="ps_q")
                    for i in range(NDC):
                        nc.tensor.matmul(
                            ps[:],
                            wq_sb[:, NV * i + 128 * cc:
                                  NV * i + 128 * (cc + 1)],
                            yt[:, SKB * i:SKB * (i + 1)],
                            start=(i == 0), stop=(i == NDC - 1))
                    nc.vector.tensor_scalar_add(
                        qT_sb[cc][:, SKB * sb:SKB * (sb + 1)], ps[:],
                        bq_sb[:, cc:cc + 1])

            def attn_scores(blk, pair, sc):
                """Row-packed concurrent score pair + one N=1024 exp."""
                sq0 = SQB * blk
                sc_ps = psc.tile([128, 2 * SQB], F32, tag="sc", name="sc_ps")
                at = attn.tile([128, 2 * SQB], FP16, tag="at", name="at")
                for hh in range(2):
                    nc.tensor.matmul(
                        sc_ps[:, SQB * hh:SQB * (hh + 1)],
                        kT_sb[pair][64 * hh:64 * (hh + 1),
                                    128 * sc:128 * (sc + 1)],
                        qT_sb[pair][64 * hh:64 * (hh + 1), sq0:sq0 + SQB],
                        tile_position=(64 * hh, 0))
                nc.scalar.activation(at[:], sc_ps[:], EXP, scale=inv_sqrt_hd)
                return at

            def attn_pv(pair, sc, at, pv_ps):
                for hh in range(2):
                    h = 2 * pair + hh
                    nc.tensor.matmul(
                        pv_ps[hh][:],
                        v_sb[sc][:, (HD + 1) * h:(HD + 1) * (h + 1)],
                        at[:, SQB * hh:SQB * (hh + 1)],
                        start=(sc == 0), stop=(sc == NKC - 1))

            def epilogue_a(blk, pair, pv_ps):
                """Drain the PV banks with cheap copies FIRST (the banks
                are a shared arena -- this window's fill work waits on
                their release), then the single-pass approx reciprocal of
                the denominator rows (moved to partition 0/1 so the
                gpsimd broadcast can source them)."""
                sq0 = SQB * blk
                for hh in range(2):
                    nc.vector.tensor_copy(
                        nv_sb[pair][64 * hh:64 * (hh + 1), sq0:sq0 + SQB],
                        pv_ps[hh][0:HD, :])
                for hh in range(2):
                    nc.vector.tensor_copy(dstg[hh][:],
                                          pv_ps[hh][HD:HD + 1, :])
                for hh in range(2):
                    nc.vector.reciprocal_approx_fast(out=dinv[hh][:],
                                                     in_=dstg[hh][:])
                    nc.vector.tensor_copy(dinvh[hh][:], dinv[hh][:])

            def epilogue_b(blk, pair, hh):
                """Broadcast 1/d over the partitions on the (idle) GPSIMD,
                then normalize in place on the DVE at 16-bit rate.  The
                broadcast fills all 128 partitions so the multiply's in1
                slice shares nv's base partition (walrus requires equal
                SBUF base partitions on TensorTensor)."""
                sq0 = SQB * blk
                nv_sl = nv_sb[pair][64 * hh:64 * (hh + 1), sq0:sq0 + SQB]
                nc.gpsimd.partition_broadcast(out_ap=rep_sb[hh][:],
                                              in_ap=dinvh[hh][:])
                nc.vector.tensor_mul(
                    nv_sl, nv_sl, rep_sb[hh][64 * hh:64 * (hh + 1), :])

            osb_box = [None]

            def outproj_unit(blk, m, dcb, pool, tag):
                """One (sq 128-chunk, 512-col) slice of the partial output
                projection, bias added on DVE eviction."""
                sq0 = SQB * blk
                if dcb == 0:
                    osb_box[0] = stream.tile([128, D], FP16, tag="o_sb",
                                             name="o_sb")
                o_sb = osb_box[0]
                o_ps = pool.tile([128, SQB], F32, tag=tag, name="o_ps")
                for pair in range(2):
                    nc.tensor.matmul(
                        o_ps[:],
                        nv_sb[pair][:, sq0 + 128 * m:sq0 + 128 * (m + 1)],
                        wo_sb[:, D * pair + 512 * dcb:
                              D * pair + 512 * (dcb + 1)],
                        start=(pair == 0), stop=(pair == 1))
                nc.vector.tensor_add(o_sb[:, 512 * dcb:512 * (dcb + 1)],
                                     o_ps[:], bo_bc[:, 512 * dcb:
                                                    512 * (dcb + 1)])
                if dcb == 1:
                    nc.sync.dma_start(
                        out=outp[sq0 + 128 * m:sq0 + 128 * (m + 1), :],
                        in_=o_sb[:])

            # ---- emission schedule ----
            # preamble: first projection slices (ACT idle anyway).  DMA
            # issue order tracks first use so the PE can start as soon as
            # the q-projection inputs land.
            yt0 = load_slice(yT, 0)
            load_wk()
            xt0 = load_slice(xT, 0)
            load_wv()
            # proj_q emits its own load for slices > 0; slice 0 reuses yt0
            for cc in range(2):
                ps = pvb.tile([128, SQB], F32, tag="pvB", name="ps_q")
                for i in range(NDC):
                    nc.tensor.matmul(
                        ps[:],
                        wq_sb[:, NV * i + 128 * cc:NV * i + 128 * (cc + 1)],
                        yt0[:, SKB * i:SKB * (i + 1)],
                        start=(i == 0), stop=(i == NDC - 1))
                nc.vector.tensor_scalar_add(
                    qT_sb[cc][:, 0:SKB], ps[:], bq_sb[:, cc:cc + 1])
            proj_kv(0, pvb, "pvB", xt=xt0)
            load_tail_params()

            prev_pv = None     # (blk, pair, tiles, pool, tag) pending epi
            for blk in range(NBLK):
                for pair in range(2):
                    pool, tag = (pva, "pvA") if pair == 0 else (pvb, "pvB")
                    fill_pool, fill_tag = ((pvb, "pvB") if pair == 0
                                           else (pva, "pvA"))
                    pv_ps = [pool.tile([128, SQB], F32, tag=tag,
                                       name=f"pv{hh}")[:HD + 1, :]
                             for hh in range(2)]
                    # keep ACT busy across the transition: two chunks of
                    # scores+exp first, then drain the previous pair
                    ats = [attn_scores(blk, pair, 0),
                           attn_scores(blk, pair, 1)]
                    pending_epi_b = None
                    if prev_pv is not None:
                        pblk, ppair, ptiles, ppool, ptag = prev_pv
                        epilogue_a(pblk, ppair, ptiles)
                        pending_epi_b = (pblk, ppair)
                    attn_pv(pair, 0, ats[0], pv_ps)
                    attn_pv(pair, 1, ats[1], pv_ps)
                    # interleaved fill work for the PE in this window;
                    # {fire_after_chunk: [emissions]}.  kT/v slice sb MUST
                    # be emitted before chunk 4*sb reads it.  outproj of
                    # blk-1 is split 4+4 units across blk's pair0 (units
                    # 0-3, after this window's epilogue_b at sc 4/6 has
                    # normalized blk-1/pair1) and pair1 (units 4-7) so
                    # each window carries ~3.5 us of outproj instead of 7.
                    fills = {}
                    if blk == 0 and pair == 0:
                        fills = {3: [lambda: proj_kv(1, pvb, "pvB")],
                                 7: [lambda: proj_kv(2, pvb, "pvB")],
                                 11: [lambda: proj_kv(3, pvb, "pvB")]}
                    elif blk == 0 and pair == 1:
                        fills = {4: [lambda: proj_q(1, pva, "pvA")],
                                 8: [lambda: proj_q(2, pva, "pvA")],
                                 12: [lambda: proj_q(3, pva, "pvA")]}
                    else:
                        lo = 0 if pair == 0 else 4
                        base = 7 if pair == 0 else 2
                        fills = {base + s: [lambda u=u: outproj_unit(
                            blk - 1, u // 2, u % 2, fill_pool, fill_tag)]
                            for s, u in enumerate(range(lo, lo + 4))}
                    for sc in range(2, NKC):
                        at = attn_scores(blk, pair, sc)
                        attn_pv(pair, sc, at, pv_ps)
                        if pending_epi_b is not None and sc in (4, 6):
                            epilogue_b(*pending_epi_b, hh=(0 if sc == 4
                                                           else 1))
                        for f in fills.get(sc, ()):
                            f()
                    prev_pv = (blk, pair, pv_ps, pool, tag)

            # tail: last pair epilogue + last block outproj
            pblk, ppair, ptiles, ppool, ptag = prev_pv
            epilogue_a(pblk, ppair, ptiles)
            epilogue_b(pblk, ppair, hh=0)
            epilogue_b(pblk, ppair, hh=1)
            for u in range(2 * (SQB // 128)):
                outproj_unit(NBLK - 1, u // 2, u % 2, pva, "pvA")

    nc.compile()
    return nc


last_results = None


def kernel(x, y, mask, Wkv, bkv, Wq, bq, Wo, bo):
    x = np.asarray(x, dtype=np.float32)
    y = np.asarray(y, dtype=np.float32)
    Wkv = np.asarray(Wkv, dtype=np.float32)
    bkv = np.asarray(bkv, dtype=np.float32)
    Wq = np.asarray(Wq, dtype=np.float32)
    bq = np.asarray(bq, dtype=np.float32)
    Wo = np.asarray(Wo, dtype=np.float32)
    bo = np.asarray(bo, dtype=np.float32)

    wkv3 = Wkv.reshape(D, H, 2 * HD)
    bv = bkv.reshape(H, 2 * HD)[:, HD:].reshape(H * HD)
    # v-bias folded into the output bias; each of the 4 partial sums per
    # batch carries bo_eff/4 so the host-side reduce reproduces bo_eff.
    bo_eff4 = ((bv @ Wo + bo) / GROUP).astype(np.float32)

    nc = build_kernel()
    in_maps = []
    for c in range(N_CORES):
        b, j = divmod(c, GROUP)
        hs = HPC * j
        f16 = np.float16
        in_maps.append({
            "yT": np.ascontiguousarray(y[b].T).astype(f16),
            "xT": np.ascontiguousarray(x[b].T).astype(f16),
            "wq": np.ascontiguousarray(
                Wq[:, HD * hs:HD * (hs + HPC)]).astype(f16),
            "wk": np.ascontiguousarray(
                wkv3[:, hs:hs + HPC, :HD].reshape(D, NV)).astype(f16),
            "wv": np.ascontiguousarray(
                wkv3[:, hs:hs + HPC, HD:].reshape(D, NV)).astype(f16),
            "wo": np.ascontiguousarray(
                Wo[HD * hs:HD * (hs + HPC), :]).astype(f16),
            "bq": np.ascontiguousarray(bq[HD * hs:HD * (hs + HPC)]),
            "bo": bo_eff4,
        })

    import os
    trace = bool(os.environ.get("KERNEL_TRACE"))
    res = run_bass_kernel_spmd(nc, in_maps, core_ids=list(range(N_CORES)),
                               trace=trace)
    global last_results
    last_results = res

    full = np.empty((B, S, D), dtype=np.float32)
    for b in range(B):
        acc = res.results[GROUP * b]["outp"].astype(np.float32)
        for j in range(1, GROUP):
            acc = acc + res.results[GROUP * b + j]["outp"].astype(np.float32)
        full[b] = acc
    return full


# revision 20
# speedup vs baseline: 1.2261x; 1.1951x over previous
"""Multi-head cross-attention kernel for Trainium2, 8 NeuronCores.

Reference computation (B=2, S=2048, D=1024, H=16, hd=64):
    kv = x @ Wkv + bkv ; q = y @ Wq + bq
    per head: s = q k^T / 8 (+ mask, all-zero per spec), a = softmax(s)
    out = concat_h(a v) @ Wo + bo

Sharding: batch (2-way) x head-groups (4 heads/core), fully collective-free.
Core c owns batch c//4 and heads 4j..4j+3 (j = c%4).  Each core computes a
PARTIAL output projection out_c = softmax(qk)v @ Wo[256-row slice] + bo/4
over the full S of its batch; the host sums the 4 partials per batch (fp16
partials; quantization ~5e-4 of partial scale, well under the 2e-2 budget).

The kernel is engine-balance driven (all matmuls fp16, fp32 PSUM):
  - ACT owns exp: 128 N=1024 ACTIVATEs ~= 147 us of irreducible work and
    paces the steady-state chunk loop at ~1.15 us/chunk.
  - PE owns the matmul stream at the observed ~2 GHz (GPIO-throttled)
    clock: projections, row-packed concurrent K=64 score pairs (two heads
    per 2-bank PSUM tile at tile_position (0,0)/(64,0)), M=65 PV matmuls
    whose extra ones-column accumulates the softmax denominator, and the
    partial outproj.  A dummy-matmul spin on memset data covers the input
    DMA preamble so HAM un-throttles before the first real matmul.
  - DVE drains PSUM and normalizes.  The softmax reciprocal uses the
    single-pass reciprocal_approx_fast custom op (~51 ULP, ~5x cheaper
    than the iterative reciprocal) on [1,512] rows.
  - GPSIMD (otherwise idle) broadcasts 1/d from one partition to the 64
    v-dim partitions via partition_broadcast, replacing the old
    ones-matmul rep broadcast on the PE.
  - Input DMA is ordered by first use (wq, y-slice0, wk, x-slice0, wv,
    then the rest); kT/v/q projection slices and outproj units are
    emitted inside the attention chunk loop to fill PE slack.  Outproj
    for blk-1 is split 4+4 units across blk's pair0/pair1 windows to
    balance PE work against the ACT envelope.

PSUM budget (8 banks): scores 2x[128,1024] double-buffer (4) + pvA (2) +
pvB (2); projections and outproj units recycle whichever pv pool had its
accumulators drained at the current window's start.
"""

import numpy as np

import concourse.bass as bass
import concourse.bacc as bacc
import concourse.mybir as mybir
from concourse.tile import TileContext
from concourse.bass_utils import run_bass_kernel_spmd

B, S, D = 2, 2048, 1024
H, HD = 16, 64
N_CORES = 8
GROUP = 4              # cores per batch group
HPC = H // GROUP       # heads per core (4)
NV = HPC * HD          # local vals rows (256)
SQB = 512              # sq block size
NBLK = S // SQB        # 4
NKC = S // 128         # 16 sk chunks
NDC = D // 128         # 8 contraction chunks
SKB = 512              # sk/sq slice size for projections

F32 = mybir.dt.float32
FP16 = mybir.dt.float16
EXP = mybir.ActivationFunctionType.Exp


def build_kernel():
    nc = bacc.Bacc("TRN2", target_bir_lowering=False, debug=False,
                   num_devices=N_CORES)

    # all inputs are pre-shuffled on the host into the exact SBUF
    # layout (partition-major, contraction-chunk-major columns) so every
    # DMA line is >= 1KB contiguous and needs no gather strides.
    # x/y are slice-major: [128, NBLK slices x NDC chunks x SKB] so a
    # half-slice load is one fully contiguous [128, 4*SKB] block (4KB
    # lines, 128 descriptors -- descriptor generation was the preamble
    # bottleneck at 1KB lines).
    yT = nc.declare_dram_parameter("yT", [128, NDC * S], FP16,
                                   isOutput=False)
    xT = nc.declare_dram_parameter("xT", [128, NDC * S], FP16,
                                   isOutput=False)
    wq = nc.declare_dram_parameter("wq", [128, NDC * NV], FP16,
                                   isOutput=False)
    wk = nc.declare_dram_parameter("wk", [128, NDC * NV], FP16,
                                   isOutput=False)
    wv = nc.declare_dram_parameter("wv", [128, NDC * NV], FP16,
                                   isOutput=False)
    wo = nc.declare_dram_parameter("wo", [128, 2 * D], FP16,
                                   isOutput=False)
    bq = nc.declare_dram_parameter("bq", [128, 2], F32, isOutput=False)
    bo = nc.declare_dram_parameter("bo", [D], F32, isOutput=False)
    outp = nc.declare_dram_parameter("outp", [S, D], FP16, isOutput=True)

    inv_sqrt_hd = float(1.0 / np.sqrt(HD))

    with TileContext(nc) as tc:
        with (
            tc.tile_pool(name="acts", bufs=1) as acts,        # persistent
            tc.tile_pool(name="wts", bufs=1) as wts,
            tc.tile_pool(name="xys", bufs=6) as xys,          # proj streaming
            tc.tile_pool(name="stream", bufs=3) as stream,
            tc.tile_pool(name="attn", bufs=4) as attn,        # exp(scores)
            tc.tile_pool(name="psc", bufs=2, space="PSUM") as psc,
            tc.tile_pool(name="pva", bufs=2, space="PSUM") as pva,
            tc.tile_pool(name="pvb", bufs=2, space="PSUM") as pvb,
        ):
            # ---- persistent tiles ----
            qT_sb = [acts.tile([128, S], FP16, tag=f"qT{i}", name=f"qT{i}")
                     for i in range(2)]
            kT_sb = [acts.tile([128, S], FP16, tag=f"kT{i}", name=f"kT{i}")
                     for i in range(2)]
            v_sb = [acts.tile([128, HPC * (HD + 1)], FP16, tag=f"v{i}",
                              name=f"v{i}") for i in range(NKC)]
            nv_sb = [acts.tile([128, S], FP16, tag=f"nv{i}", name=f"nv{i}")
                     for i in range(2)]
            # softmax denominator staging: per head, [1,512] rows at
            # partition 0 (copied cross-partition off the PV banks' row 64)
            dstg = [acts.tile([1, SQB], F32, tag=f"dstg{h}", name=f"dstg{h}")
                    for h in range(2)]
            dinv = [acts.tile([1, SQB], F32, tag=f"dinv{h}", name=f"dinv{h}")
                    for h in range(2)]
            dinvh = [acts.tile([1, SQB], FP16, tag=f"dinvh{h}", name=f"dinvh{h}")
                     for h in range(2)]
            rep_sb = [acts.tile([128, SQB], FP16, tag=f"rep{h}",
                                name=f"rep{h}") for h in range(2)]
            bq_sb = acts.tile([128, 2], F32, tag="bq")
            bo_bc = acts.tile([128, D], F32, tag="bo_bc")
            warm = acts.tile([1, 8], F32, tag="warm")
            wones = acts.tile([64, 256], FP16, tag="wones")

            # preload the exp table set + HAM warmup while input DMA streams:
            # dummy matmuls keep the PE busy through the weight/slice0 DMA
            # wait so the activity throttle lifts before the first real
            # matmul (a >3.4us PE-idle gap re-throttles to half clock).
            nc.vector.memset(warm[:], 0.0)
            nc.scalar.activation(warm[:], warm[:], EXP)
            nc.vector.memset(wones[:], 0.0)
            warm_ps = pva.tile([128, SQB], F32, tag="pvA", name="warm_ps")
            for _ in range(40):
                nc.tensor.matmul(warm_ps[:, 0:256], wones[:, 0:128],
                                 wones[:, 0:256])

            # weights + first slices, one dma_start per tensor, ordered by
            # first use: wq -> y slice0 (q proj), wk -> x slice0 (k proj),
            # wv (v proj).  bo/wo deferred past the preamble.
            wk_sb = wts.tile([128, NDC * NV], FP16, tag="wk")
            wv_sb = wts.tile([128, NDC * NV], FP16, tag="wv")
            wq_sb = wts.tile([128, NDC * NV], FP16, tag="wq")
            wo_sb = wts.tile([128, 2 * D], FP16, tag="wo")
            nc.scalar.dma_start(out=bq_sb[:], in_=bq[:, :])
            nc.scalar.dma_start(out=wq_sb[:], in_=wq[:, :])

            def load_tail_params():
                nc.scalar.dma_start(
                    out=bo_bc[:], in_=bo[None, :].to_broadcast((128, D)))
                nc.scalar.dma_start(out=wo_sb[:], in_=wo[:, :])

            # ---- emission helpers ----
            def load_slice(src, sb):
                """Slice sb as two half-tiles (chunks 0-3 / 4-7) so the
                first accumulation matmuls can start at half-DMA.  The
                slice-major DRAM layout makes each half one contiguous
                [128, 4*SKB] block."""
                ts = []
                for half in range(2):
                    t = xys.tile([128, 4 * SKB], FP16, tag="xys",
                                 name="xys")
                    off = (2 * sb + half) * 4 * SKB
                    nc.sync.dma_start(out=t[:],
                                      in_=src[:, off:off + 4 * SKB])
                    ts.append(t)
                return ts

            def xch(ts, i):
                return ts[i // 4][:, SKB * (i % 4):SKB * (i % 4 + 1)]

            def load_wk():
                nc.scalar.dma_start(out=wk_sb[:], in_=wk[:, :])

            def load_wv():
                nc.scalar.dma_start(out=wv_sb[:], in_=wv[:, :])

            def proj_k_cc(sb, cc, xt, pool, tag):
                ps = pool.tile([128, SQB], F32, tag=tag, name="ps_k")
                for i in range(NDC):
                    nc.tensor.matmul(
                        ps[:],
                        wk_sb[:, NV * i + 128 * cc:NV * i + 128 * (cc + 1)],
                        xch(xt, i),
                        start=(i == 0), stop=(i == NDC - 1))
                nc.vector.tensor_copy(
                    kT_sb[cc][:, SKB * sb:SKB * (sb + 1)], ps[:])

            def proj_v_sc4(sb, sc4, xt, pool, tag):
                ps = pool.tile([128, SQB], F32, tag=tag, name="ps_v")
                ps = ps[:, :NV]
                for i in range(NDC):
                    nc.tensor.matmul(
                        ps[:],
                        xch(xt, i)[:, 128 * sc4:128 * (sc4 + 1)],
                        wv_sb[:, NV * i:NV * (i + 1)],
                        start=(i == 0), stop=(i == NDC - 1))
                ks = sb * (SKB // 128) + sc4
                # ones columns for the in-matmul denominator row
                nc.vector.memset(
                    v_sb[ks][:].rearrange("p (h c) -> p h c",
                                          c=HD + 1)[:, :, HD:HD + 1],
                    1.0)
                nc.vector.tensor_copy(
                    v_sb[ks][:].rearrange("p (h c) -> p h c",
                                          c=HD + 1)[:, :, 0:HD],
                    ps[:].rearrange("p (h c) -> p h c", c=HD))

            def proj_q_cc(sb, cc, yt, pool, tag):
                ps = pool.tile([128, SQB], F32, tag=tag, name="ps_q")
                for i in range(NDC):
                    nc.tensor.matmul(
                        ps[:],
                        wq_sb[:, NV * i + 128 * cc:NV * i + 128 * (cc + 1)],
                        xch(yt, i),
                        start=(i == 0), stop=(i == NDC - 1))
                nc.vector.tensor_scalar_add(
                    qT_sb[cc][:, SKB * sb:SKB * (sb + 1)], ps[:],
                    bq_sb[:, cc:cc + 1])

            def attn_scores(blk, pair, sc):
                """Row-packed concurrent score pair + one N=1024 exp."""
                sq0 = SQB * blk
                sc_ps = psc.tile([128, 2 * SQB], F32, tag="sc", name="sc_ps")
                at = attn.tile([128, 2 * SQB], FP16, tag="at", name="at")
                for hh in range(2):
                    nc.tensor.matmul(
                        sc_ps[:, SQB * hh:SQB * (hh + 1)],
                        kT_sb[pair][64 * hh:64 * (hh + 1),
                                    128 * sc:128 * (sc + 1)],
                        qT_sb[pair][64 * hh:64 * (hh + 1), sq0:sq0 + SQB],
                        tile_position=(64 * hh, 0))
                nc.scalar.activation(at[:], sc_ps[:], EXP, scale=inv_sqrt_hd)
                return at

            def attn_pv(pair, sc, at, pv_ps):
                for hh in range(2):
                    h = 2 * pair + hh
                    nc.tensor.matmul(
                        pv_ps[hh][:],
                        v_sb[sc][:, (HD + 1) * h:(HD + 1) * (h + 1)],
                        at[:, SQB * hh:SQB * (hh + 1)],
                        start=(sc == 0), stop=(sc == NKC - 1))

            def epilogue_a(blk, pair, pv_ps, tail=False):
                """Drain the PV banks with cheap copies FIRST (the banks
                are a shared arena -- this window's fill work waits on
                their release), then the single-pass approx reciprocal of
                the denominator rows, read straight off the PSUM row 64
                into partition 0 (DVE handles the partition/space move).
                In the tail the order flips: the d-chain is the critical
                path to the last outproj, bank release doesn't matter."""
                sq0 = SQB * blk

                def drains():
                    for hh in range(2):
                        nc.vector.tensor_copy(
                            nv_sb[pair][64 * hh:64 * (hh + 1),
                                        sq0:sq0 + SQB],
                            pv_ps[hh][0:HD, :])

                def dchain():
                    for hh in range(2):
                        nc.vector.tensor_copy(dstg[hh][:],
                                              pv_ps[hh][HD:HD + 1, :])
                    for hh in range(2):
                        nc.vector.reciprocal_approx_fast(
                            out=dinv[hh][:], in_=dstg[hh][:])
                        nc.vector.tensor_copy(dinvh[hh][:], dinv[hh][:])

                if tail:
                    dchain()
                    drains()
                else:
                    drains()
                    dchain()

            def epilogue_b(blk, pair, hh):
                """Broadcast 1/d over the partitions on the (idle) GPSIMD,
                then normalize in place on the DVE at 16-bit rate.  The
                broadcast fills all 128 partitions so the multiply's in1
                slice shares nv's base partition (walrus requires equal
                SBUF base partitions on TensorTensor)."""
                sq0 = SQB * blk
                nv_sl = nv_sb[pair][64 * hh:64 * (hh + 1), sq0:sq0 + SQB]
                nc.gpsimd.partition_broadcast(out_ap=rep_sb[hh][:],
                                              in_ap=dinvh[hh][:])
                nc.vector.tensor_mul(
                    nv_sl, nv_sl, rep_sb[hh][64 * hh:64 * (hh + 1), :])

            osb_box = [None]

            def outproj_unit(blk, m, dcb, pool, tag, dma_halves=False):
                """One (sq 128-chunk, 512-col) slice of the partial output
                projection, bias added on DVE eviction."""
                sq0 = SQB * blk
                if dcb == 0:
                    osb_box[0] = stream.tile([128, D], FP16, tag="o_sb",
                                             name="o_sb")
                o_sb = osb_box[0]
                o_ps = pool.tile([128, SQB], F32, tag=tag, name="o_ps")
                for pair in range(2):
                    nc.tensor.matmul(
                        o_ps[:],
                        nv_sb[pair][:, sq0 + 128 * m:sq0 + 128 * (m + 1)],
                        wo_sb[:, D * pair + 512 * dcb:
                              D * pair + 512 * (dcb + 1)],
                        start=(pair == 0), stop=(pair == 1))
                nc.vector.tensor_add(o_sb[:, 512 * dcb:512 * (dcb + 1)],
                                     o_ps[:], bo_bc[:, 512 * dcb:
                                                    512 * (dcb + 1)])
                if dma_halves:
                    nc.sync.dma_start(
                        out=outp[sq0 + 128 * m:sq0 + 128 * (m + 1),
                                 512 * dcb:512 * (dcb + 1)],
                        in_=o_sb[:, 512 * dcb:512 * (dcb + 1)])
                elif dcb == 1:
                    nc.sync.dma_start(
                        out=outp[sq0 + 128 * m:sq0 + 128 * (m + 1), :],
                        in_=o_sb[:])

            # ---- emission schedule ----
            # preamble: only the projection units the first attention
            # window needs up front (q cc0, k cc0 of slice 0); everything
            # else streams in as chunk-loop fills so the EXP pipeline
            # starts ~15 us earlier.  DMA issue order tracks first use.
            yt0 = load_slice(yT, 0)
            load_wk()
            xt0 = load_slice(xT, 0)
            load_wv()
            proj_q_cc(0, 0, yt0, pvb, "pvB")
            proj_k_cc(0, 0, xt0, pvb, "pvB")
            load_tail_params()

            xts = {0: xt0}
            yts = {0: yt0}

            def kv_batch(sb):
                """k (both cc) + first v part of slice sb, with its DMA."""
                xts[sb] = load_slice(xT, sb)
                proj_k_cc(sb, 0, xts[sb], pvb, "pvB")
                proj_k_cc(sb, 1, xts[sb], pvb, "pvB")
                proj_v_sc4(sb, 0, xts[sb], pvb, "pvB")

            def q_batch(sb):
                yts[sb] = load_slice(yT, sb)
                proj_q_cc(sb, 0, yts[sb], pva, "pvA")
                proj_q_cc(sb, 1, yts[sb], pva, "pvA")

            prev_pv = None     # (blk, pair, tiles) pending epilogue
            for blk in range(NBLK):
                for pair in range(2):
                    pool, tag = (pva, "pvA") if pair == 0 else (pvb, "pvB")
                    fill_pool, fill_tag = ((pvb, "pvB") if pair == 0
                                           else (pva, "pvA"))
                    pv_ps = [pool.tile([128, SQB], F32, tag=tag,
                                       name=f"pv{hh}")[:HD + 1, :]
                             for hh in range(2)]
                    if blk == 0 and pair == 0:
                        # first v parts BEFORE the first scores: the PE
                        # semaphore increments are batched, so anything
                        # emitted between the scores and their EXP
                        # consumers stretches the EXP's wait threshold
                        proj_v_sc4(0, 0, xt0, pvb, "pvB")
                        proj_v_sc4(0, 1, xt0, pvb, "pvB")
                    # keep ACT busy across the transition: two chunks of
                    # scores+exp first, then drain the previous pair
                    ats = [attn_scores(blk, pair, 0),
                           attn_scores(blk, pair, 1)]
                    pending_epi_b = None
                    if prev_pv is not None:
                        pblk, ppair, ptiles = prev_pv
                        epilogue_a(pblk, ppair, ptiles)
                        pending_epi_b = (pblk, ppair)
                    attn_pv(pair, 0, ats[0], pv_ps)
                    attn_pv(pair, 1, ats[1], pv_ps)
                    if blk == 0 and pair == 0:
                        # pv(2) consumes this at the first loop iteration
                        proj_v_sc4(0, 2, xt0, pvb, "pvB")
                    # interleaved fill work for the PE in this window;
                    # {fire_after_chunk: [emissions]}.  kT/v slice sb MUST
                    # be emitted before chunk 4*sb reads it.  outproj of
                    # blk-1 is split 4+4 units across blk's pair0 (units
                    # 0-3, after this window's epilogue_b at sc 4/6 has
                    # normalized blk-1/pair1) and pair1 (units 4-7) so
                    # each window carries ~3.5 us of outproj instead of 7.
                    if blk == 0 and pair == 0:
                        # fills(sc) emit AFTER pv(sc): v_sb[k] must be
                        # emitted at fills(k-1) or earlier; kT slice sb
                        # before scores(4*sb) i.e. fills(4*sb-1).
                        fills = {
                            2: [lambda: proj_v_sc4(0, 3, xt0, pvb, "pvB")],
                            3: [lambda: kv_batch(1),
                                lambda: proj_v_sc4(1, 1, xts[1], pvb,
                                                   "pvB")],
                            5: [lambda: proj_v_sc4(1, 2, xts[1], pvb,
                                                   "pvB"),
                                lambda: proj_v_sc4(1, 3, xts[1], pvb,
                                                   "pvB")],
                            6: [lambda: proj_k_cc(0, 1, xt0, pvb, "pvB"),
                                lambda: proj_q_cc(0, 1, yt0, pvb, "pvB")],
                            7: [lambda: kv_batch(2),
                                lambda: proj_v_sc4(2, 1, xts[2], pvb,
                                                   "pvB")],
                            9: [lambda: proj_v_sc4(2, 2, xts[2], pvb,
                                                   "pvB")],
                            10: [lambda: proj_v_sc4(2, 3, xts[2], pvb,
                                                    "pvB")],
                            11: [lambda: kv_batch(3),
                                 lambda: proj_v_sc4(3, 1, xts[3], pvb,
                                                    "pvB")],
                            13: [lambda: proj_v_sc4(3, 2, xts[3], pvb,
                                                    "pvB")],
                            14: [lambda: proj_v_sc4(3, 3, xts[3], pvb,
                                                    "pvB")],
                        }
                    elif blk == 0 and pair == 1:
                        fills = {3: [lambda: q_batch(1)],
                                 7: [lambda: q_batch(2)],
                                 11: [lambda: q_batch(3)]}
                    else:
                        # 5 units in pair0 windows (they run ~1us under
                        # the ACT envelope), 3 in the outproj-heavier
                        # pair1 windows
                        lo, n = (0, 5) if pair == 0 else (5, 3)
                        base = 7 if pair == 0 else 2
                        fills = {base + s: [lambda u=u: outproj_unit(
                            blk - 1, u // 2, u % 2, fill_pool, fill_tag)]
                            for s, u in enumerate(range(lo, lo + n))}
                    for sc in range(2, NKC):
                        at = attn_scores(blk, pair, sc)
                        attn_pv(pair, sc, at, pv_ps)
                        if pending_epi_b is not None and sc in (4, 6):
                            epilogue_b(*pending_epi_b, hh=(0 if sc == 4
                                                           else 1))
                        for f in fills.get(sc, ()):
                            f()
                    prev_pv = (blk, pair, pv_ps)

            # tail: last pair epilogue (d-chain first -- it gates the
            # final outproj).  The first 4 units' pair0 matmuls depend
            # only on the long-normalized nv[pair0], so they stream on
            # the PE while the DVE runs the d-chain; their pair1 halves
            # and the remaining units follow the normalize muls.
            pblk, ppair, ptiles = prev_pv
            epilogue_a(pblk, ppair, ptiles, tail=True)
            sq0 = SQB * (NBLK - 1)
            tail_ps = []
            tail_osb = []
            for u in range(4):
                pool, tag = (pva, "pvA") if u % 2 == 0 else (pvb, "pvB")
                m, dcb = u // 2, u % 2
                if dcb == 0:
                    osb_box[0] = stream.tile([128, D], FP16, tag="o_sb",
                                             name="o_sb")
                tail_osb.append(osb_box[0])
                o_ps = pool.tile([128, SQB], F32, tag=tag, name="o_ps")
                nc.tensor.matmul(
                    o_ps[:],
                    nv_sb[0][:, sq0 + 128 * m:sq0 + 128 * (m + 1)],
                    wo_sb[:, 512 * dcb:512 * (dcb + 1)],
                    start=True, stop=False)
                tail_ps.append(o_ps)
            epilogue_b(pblk, ppair, hh=0)
            epilogue_b(pblk, ppair, hh=1)
            for u in range(4):
                m, dcb = u // 2, u % 2
                o_ps, o_sb = tail_ps[u], tail_osb[u]
                nc.tensor.matmul(
                    o_ps[:],
                    nv_sb[1][:, sq0 + 128 * m:sq0 + 128 * (m + 1)],
                    wo_sb[:, D + 512 * dcb:D + 512 * (dcb + 1)],
                    start=False, stop=True)
                nc.vector.tensor_add(o_sb[:, 512 * dcb:512 * (dcb + 1)],
                                     o_ps[:], bo_bc[:, 512 * dcb:
                                                    512 * (dcb + 1)])
                nc.sync.dma_start(
                    out=outp[sq0 + 128 * m:sq0 + 128 * (m + 1),
                             512 * dcb:512 * (dcb + 1)],
                    in_=o_sb[:, 512 * dcb:512 * (dcb + 1)])
            for u in range(4, 2 * (SQB // 128)):
                pool, tag = (pva, "pvA") if u % 2 == 0 else (pvb, "pvB")
                outproj_unit(NBLK - 1, u // 2, u % 2, pool, tag,
                             dma_halves=True)

    nc.compile()
    return nc


last_results = None


def kernel(x, y, mask, Wkv, bkv, Wq, bq, Wo, bo):
    x = np.asarray(x, dtype=np.float32)
    y = np.asarray(y, dtype=np.float32)
    Wkv = np.asarray(Wkv, dtype=np.float32)
    bkv = np.asarray(bkv, dtype=np.float32)
    Wq = np.asarray(Wq, dtype=np.float32)
    bq = np.asarray(bq, dtype=np.float32)
    Wo = np.asarray(Wo, dtype=np.float32)
    bo = np.asarray(bo, dtype=np.float32)

    wkv3 = Wkv.reshape(D, H, 2 * HD)
    bv = bkv.reshape(H, 2 * HD)[:, HD:].reshape(H * HD)
    # v-bias folded into the output bias; each of the 4 partial sums per
    # batch carries bo_eff/4 so the host-side reduce reproduces bo_eff.
    bo_eff4 = ((bv @ Wo + bo) / GROUP).astype(np.float32)

    def chunk_major(a):
        """[D, M] -> [128, NDC*M]: SBUF layout, contraction-chunk-major."""
        d, m = a.shape
        return np.ascontiguousarray(
            a.reshape(d // 128, 128, m).transpose(1, 0, 2).reshape(
                128, (d // 128) * m))

    def slice_major(a):
        """[D, S] -> [128, NBLK*NDC*SKB]: per 512-col slice, chunk-major
        contiguous."""
        d, s = a.shape
        return np.ascontiguousarray(
            a.reshape(d // 128, 128, s // SKB, SKB)
            .transpose(1, 2, 0, 3).reshape(128, s * (d // 128)))

    nc = build_kernel()
    in_maps = []
    for c in range(N_CORES):
        b, j = divmod(c, GROUP)
        hs = HPC * j
        f16 = np.float16
        in_maps.append({
            "yT": slice_major(y[b].T).astype(f16),
            "xT": slice_major(x[b].T).astype(f16),
            "wq": chunk_major(Wq[:, HD * hs:HD * (hs + HPC)]).astype(f16),
            "wk": chunk_major(
                wkv3[:, hs:hs + HPC, :HD].reshape(D, NV)).astype(f16),
            "wv": chunk_major(
                wkv3[:, hs:hs + HPC, HD:].reshape(D, NV)).astype(f16),
            "wo": chunk_major(
                Wo[HD * hs:HD * (hs + HPC), :]).astype(f16),
            "bq": np.ascontiguousarray(
                bq[HD * hs:HD * (hs + HPC)].reshape(2, 128).T),
            "bo": bo_eff4,
        })

    import os
    trace = bool(os.environ.get("KERNEL_TRACE"))
    res = run_bass_kernel_spmd(nc, in_maps, core_ids=list(range(N_CORES)),
                               trace=trace)
    global last_results
    last_results = res

    full = np.empty((B, S, D), dtype=np.float32)
    for b in range(B):
        acc = res.results[GROUP * b]["outp"].astype(np.float32)
        for j in range(1, GROUP):
            acc = acc + res.results[GROUP * b + j]["outp"].astype(np.float32)
        full[b] = acc
    return full


# revision 22
# speedup vs baseline: 1.2510x; 1.0203x over previous
"""Multi-head cross-attention kernel for Trainium2, 8 NeuronCores.

Reference computation (B=2, S=2048, D=1024, H=16, hd=64):
    kv = x @ Wkv + bkv ; q = y @ Wq + bq
    per head: s = q k^T / 8 (+ mask, all-zero per spec), a = softmax(s)
    out = concat_h(a v) @ Wo + bo

Sharding: batch (2-way) x head-groups (4 heads/core), fully collective-free.
Core c owns batch c//4 and heads 4j..4j+3 (j = c%4).  Each core computes a
PARTIAL output projection out_c = softmax(qk)v @ Wo[256-row slice] + bo/4
over the full S of its batch; the host sums the 4 partials per batch (fp16
partials; quantization ~5e-4 of partial scale, well under the 2e-2 budget).

The kernel is engine-balance driven (all matmuls fp16, fp32 PSUM):
  - ACT owns exp: 128 N=1024 ACTIVATEs ~= 147 us of irreducible work and
    paces the steady-state chunk loop at ~1.15 us/chunk.
  - PE owns the matmul stream at the observed ~2 GHz (GPIO-throttled)
    clock: projections, row-packed concurrent K=64 score pairs (two heads
    per 2-bank PSUM tile at tile_position (0,0)/(64,0)), M=65 PV matmuls
    whose extra ones-column accumulates the softmax denominator, and the
    partial outproj.  A dummy-matmul spin on memset data covers the input
    DMA preamble so HAM un-throttles before the first real matmul.
  - DVE drains PSUM and normalizes.  The softmax reciprocal uses the
    single-pass reciprocal_approx_fast custom op (~51 ULP, ~5x cheaper
    than the iterative reciprocal) on [1,512] rows.
  - GPSIMD (otherwise idle) broadcasts 1/d from one partition to the 64
    v-dim partitions via partition_broadcast, replacing the old
    ones-matmul rep broadcast on the PE.
  - Input DMA is ordered by first use (wq, y-slice0, wk, x-slice0, wv,
    then the rest); kT/v/q projection slices and outproj units are
    emitted inside the attention chunk loop to fill PE slack.  Outproj
    for blk-1 is split 4+4 units across blk's pair0/pair1 windows to
    balance PE work against the ACT envelope.

PSUM budget (8 banks): scores 2x[128,1024] double-buffer (4) + pvA (2) +
pvB (2); projections and outproj units recycle whichever pv pool had its
accumulators drained at the current window's start.
"""

import numpy as np

import concourse.bass as bass
import concourse.bacc as bacc
import concourse.mybir as mybir
from concourse.tile import TileContext
from concourse.bass_utils import run_bass_kernel_spmd

B, S, D = 2, 2048, 1024
H, HD = 16, 64
N_CORES = 8
GROUP = 4              # cores per batch group
HPC = H // GROUP       # heads per core (4)
NV = HPC * HD          # local vals rows (256)
SQB = 512              # sq block size
NBLK = S // SQB        # 4
NKC = S // 128         # 16 sk chunks
NDC = D // 128         # 8 contraction chunks
SKB = 512              # sk/sq slice size for projections

F32 = mybir.dt.float32
FP16 = mybir.dt.float16
EXP = mybir.ActivationFunctionType.Exp


def build_kernel():
    nc = bacc.Bacc("TRN2", target_bir_lowering=False, debug=False,
                   num_devices=N_CORES)

    # all inputs are pre-shuffled on the host into the exact SBUF
    # layout (partition-major, contraction-chunk-major columns) so every
    # DMA line is >= 1KB contiguous and needs no gather strides.
    # x/y are slice-major: [128, NBLK slices x NDC chunks x SKB] so a
    # half-slice load is one fully contiguous [128, 4*SKB] block (4KB
    # lines, 128 descriptors -- descriptor generation was the preamble
    # bottleneck at 1KB lines).
    yT = nc.declare_dram_parameter("yT", [128, NDC * S], FP16,
                                   isOutput=False)
    xT = nc.declare_dram_parameter("xT", [128, NDC * S], FP16,
                                   isOutput=False)
    wq = nc.declare_dram_parameter("wq", [128, NDC * NV], FP16,
                                   isOutput=False)
    wk = nc.declare_dram_parameter("wk", [128, NDC * NV], FP16,
                                   isOutput=False)
    wv = nc.declare_dram_parameter("wv", [128, NDC * NV], FP16,
                                   isOutput=False)
    wo = nc.declare_dram_parameter("wo", [128, 2 * D], FP16,
                                   isOutput=False)
    bq = nc.declare_dram_parameter("bq", [128, 2], F32, isOutput=False)
    bo = nc.declare_dram_parameter("bo", [D], F32, isOutput=False)
    outp = nc.declare_dram_parameter("outp", [S, D], FP16, isOutput=True)

    inv_sqrt_hd = float(1.0 / np.sqrt(HD))

    with TileContext(nc) as tc:
        with (
            tc.tile_pool(name="acts", bufs=1) as acts,        # persistent
            tc.tile_pool(name="wts", bufs=1) as wts,
            tc.tile_pool(name="xys", bufs=6) as xys,          # proj streaming
            tc.tile_pool(name="stream", bufs=3) as stream,
            tc.tile_pool(name="attn", bufs=4) as attn,        # exp(scores)
            tc.tile_pool(name="psc", bufs=2, space="PSUM") as psc,
            tc.tile_pool(name="pva", bufs=2, space="PSUM") as pva,
            tc.tile_pool(name="pvb", bufs=2, space="PSUM") as pvb,
        ):
            # ---- persistent tiles ----
            qT_sb = [acts.tile([128, S], FP16, tag=f"qT{i}", name=f"qT{i}")
                     for i in range(2)]
            kT_sb = [acts.tile([128, S], FP16, tag=f"kT{i}", name=f"kT{i}")
                     for i in range(2)]
            v_sb = [acts.tile([128, HPC * (HD + 1)], FP16, tag=f"v{i}",
                              name=f"v{i}") for i in range(NKC)]
            nv_sb = [acts.tile([128, S], FP16, tag=f"nv{i}", name=f"nv{i}")
                     for i in range(2)]
            # softmax denominator staging: per head, [1,512] rows at
            # partition 0 (copied cross-partition off the PV banks' row 64)
            dstg = [acts.tile([1, SQB], F32, tag=f"dstg{h}", name=f"dstg{h}")
                    for h in range(2)]
            dinv = [acts.tile([1, SQB], F32, tag=f"dinv{h}", name=f"dinv{h}")
                    for h in range(2)]
            dinvh = [acts.tile([1, SQB], FP16, tag=f"dinvh{h}", name=f"dinvh{h}")
                     for h in range(2)]
            rep_sb = [acts.tile([128, SQB], FP16, tag=f"rep{h}",
                                name=f"rep{h}") for h in range(2)]
            bq_sb = acts.tile([128, 2], F32, tag="bq")
            bo_bc = acts.tile([128, D], F32, tag="bo_bc")
            warm = acts.tile([1, 8], F32, tag="warm")
            wones = acts.tile([64, 256], FP16, tag="wones")

            # preload the exp table set + HAM warmup while input DMA streams:
            # dummy matmuls keep the PE busy through the weight/slice0 DMA
            # wait so the activity throttle lifts before the first real
            # matmul (a >3.4us PE-idle gap re-throttles to half clock).
            nc.vector.memset(warm[:], 0.0)
            nc.scalar.activation(warm[:], warm[:], EXP)
            nc.vector.memset(wones[:], 0.0)
            warm_ps = pva.tile([128, SQB], F32, tag="pvA", name="warm_ps")
            for _ in range(40):
                nc.tensor.matmul(warm_ps[:, 0:256], wones[:, 0:128],
                                 wones[:, 0:256])

            # weights + first slices, one dma_start per tensor, ordered by
            # first use: wq -> y slice0 (q proj), wk -> x slice0 (k proj),
            # wv (v proj).  bo/wo deferred past the preamble.
            wk_sb = wts.tile([128, NDC * NV], FP16, tag="wk")
            wv_sb = wts.tile([128, NDC * NV], FP16, tag="wv")
            wq_sb = wts.tile([128, NDC * NV], FP16, tag="wq")
            wo_sb = wts.tile([128, 2 * D], FP16, tag="wo")
            nc.scalar.dma_start(out=bq_sb[:], in_=bq[:, :])
            nc.scalar.dma_start(out=wq_sb[:], in_=wq[:, :])

            def load_tail_params():
                nc.scalar.dma_start(
                    out=bo_bc[:], in_=bo[None, :].to_broadcast((128, D)))
                nc.scalar.dma_start(out=wo_sb[:], in_=wo[:, :])

            # ---- emission helpers ----
            def load_slice(src, sb):
                """Slice sb as two half-tiles (chunks 0-3 / 4-7) so the
                first accumulation matmuls can start at half-DMA.  The
                slice-major DRAM layout makes each half one contiguous
                [128, 4*SKB] block."""
                ts = []
                for half in range(2):
                    t = xys.tile([128, 4 * SKB], FP16, tag="xys",
                                 name="xys")
                    off = (2 * sb + half) * 4 * SKB
                    nc.sync.dma_start(out=t[:],
                                      in_=src[:, off:off + 4 * SKB])
                    ts.append(t)
                return ts

            def xch(ts, i):
                return ts[i // 4][:, SKB * (i % 4):SKB * (i % 4 + 1)]

            def load_wk():
                nc.scalar.dma_start(out=wk_sb[:], in_=wk[:, :])

            def load_wv():
                nc.scalar.dma_start(out=wv_sb[:], in_=wv[:, :])

            def proj_k_cc(sb, cc, xt, pool, tag):
                ps = pool.tile([128, SQB], F32, tag=tag, name="ps_k")
                for i in range(NDC):
                    nc.tensor.matmul(
                        ps[:],
                        wk_sb[:, NV * i + 128 * cc:NV * i + 128 * (cc + 1)],
                        xch(xt, i),
                        start=(i == 0), stop=(i == NDC - 1))
                nc.vector.tensor_copy(
                    kT_sb[cc][:, SKB * sb:SKB * (sb + 1)], ps[:])

            def proj_v_sc4(sb, sc4, xt, pool, tag):
                ps = pool.tile([128, SQB], F32, tag=tag, name="ps_v")
                ps = ps[:, :NV]
                for i in range(NDC):
                    nc.tensor.matmul(
                        ps[:],
                        xch(xt, i)[:, 128 * sc4:128 * (sc4 + 1)],
                        wv_sb[:, NV * i:NV * (i + 1)],
                        start=(i == 0), stop=(i == NDC - 1))
                ks = sb * (SKB // 128) + sc4
                # ones columns for the in-matmul denominator row
                nc.vector.memset(
                    v_sb[ks][:].rearrange("p (h c) -> p h c",
                                          c=HD + 1)[:, :, HD:HD + 1],
                    1.0)
                nc.vector.tensor_copy(
                    v_sb[ks][:].rearrange("p (h c) -> p h c",
                                          c=HD + 1)[:, :, 0:HD],
                    ps[:].rearrange("p (h c) -> p h c", c=HD))

            def proj_q_cc(sb, cc, yt, pool, tag):
                ps = pool.tile([128, SQB], F32, tag=tag, name="ps_q")
                for i in range(NDC):
                    nc.tensor.matmul(
                        ps[:],
                        wq_sb[:, NV * i + 128 * cc:NV * i + 128 * (cc + 1)],
                        xch(yt, i),
                        start=(i == 0), stop=(i == NDC - 1))
                nc.vector.tensor_scalar_add(
                    qT_sb[cc][:, SKB * sb:SKB * (sb + 1)], ps[:],
                    bq_sb[:, cc:cc + 1])

            def attn_scores(blk, pair, sc):
                """Row-packed concurrent score pair + one N=1024 exp."""
                sq0 = SQB * blk
                sc_ps = psc.tile([128, 2 * SQB], F32, tag="sc", name="sc_ps")
                at = attn.tile([128, 2 * SQB], FP16, tag="at", name="at")
                for hh in range(2):
                    nc.tensor.matmul(
                        sc_ps[:, SQB * hh:SQB * (hh + 1)],
                        kT_sb[pair][64 * hh:64 * (hh + 1),
                                    128 * sc:128 * (sc + 1)],
                        qT_sb[pair][64 * hh:64 * (hh + 1), sq0:sq0 + SQB],
                        tile_position=(64 * hh, 0))
                nc.scalar.activation(at[:], sc_ps[:], EXP, scale=inv_sqrt_hd)
                return at

            def attn_pv(pair, sc, at, pv_ps):
                for hh in range(2):
                    h = 2 * pair + hh
                    nc.tensor.matmul(
                        pv_ps[hh][:],
                        v_sb[sc][:, (HD + 1) * h:(HD + 1) * (h + 1)],
                        at[:, SQB * hh:SQB * (hh + 1)],
                        start=(sc == 0), stop=(sc == NKC - 1))

            def epilogue_a(blk, pair, pv_ps, tail=False):
                """Drain the PV banks with cheap copies FIRST (the banks
                are a shared arena -- this window's fill work waits on
                their release), then the single-pass approx reciprocal of
                the denominator rows, read straight off the PSUM row 64
                into partition 0 (DVE handles the partition/space move).
                In the tail the order flips: the d-chain is the critical
                path to the last outproj, bank release doesn't matter."""
                sq0 = SQB * blk

                def drains():
                    for hh in range(2):
                        nc.vector.tensor_copy(
                            nv_sb[pair][64 * hh:64 * (hh + 1),
                                        sq0:sq0 + SQB],
                            pv_ps[hh][0:HD, :])

                def dchain():
                    for hh in range(2):
                        nc.vector.tensor_copy(dstg[hh][:],
                                              pv_ps[hh][HD:HD + 1, :])
                    for hh in range(2):
                        nc.vector.reciprocal_approx_fast(
                            out=dinv[hh][:], in_=dstg[hh][:])
                        nc.vector.tensor_copy(dinvh[hh][:], dinv[hh][:])

                if tail:
                    dchain()
                    drains()
                else:
                    drains()
                    dchain()

            def epilogue_b(blk, pair, hh):
                """Broadcast 1/d over the partitions on the (idle) GPSIMD,
                then normalize in place on the DVE at 16-bit rate.  The
                broadcast fills all 128 partitions so the multiply's in1
                slice shares nv's base partition (walrus requires equal
                SBUF base partitions on TensorTensor)."""
                sq0 = SQB * blk
                nv_sl = nv_sb[pair][64 * hh:64 * (hh + 1), sq0:sq0 + SQB]
                nc.gpsimd.partition_broadcast(out_ap=rep_sb[hh][:],
                                              in_ap=dinvh[hh][:])
                nc.vector.tensor_mul(
                    nv_sl, nv_sl, rep_sb[hh][64 * hh:64 * (hh + 1), :])

            osb_box = [None]

            def outproj_unit(blk, m, dcb, pool, tag, dma_halves=False):
                """One (sq 128-chunk, 512-col) slice of the partial output
                projection, bias added on DVE eviction."""
                sq0 = SQB * blk
                if dcb == 0:
                    osb_box[0] = stream.tile([128, D], FP16, tag="o_sb",
                                             name="o_sb")
                o_sb = osb_box[0]
                o_ps = pool.tile([128, SQB], F32, tag=tag, name="o_ps")
                for pair in range(2):
                    nc.tensor.matmul(
                        o_ps[:],
                        nv_sb[pair][:, sq0 + 128 * m:sq0 + 128 * (m + 1)],
                        wo_sb[:, D * pair + 512 * dcb:
                              D * pair + 512 * (dcb + 1)],
                        start=(pair == 0), stop=(pair == 1))
                nc.vector.tensor_add(o_sb[:, 512 * dcb:512 * (dcb + 1)],
                                     o_ps[:], bo_bc[:, 512 * dcb:
                                                    512 * (dcb + 1)])
                if dma_halves:
                    nc.sync.dma_start(
                        out=outp[sq0 + 128 * m:sq0 + 128 * (m + 1),
                                 512 * dcb:512 * (dcb + 1)],
                        in_=o_sb[:, 512 * dcb:512 * (dcb + 1)])
                elif dcb == 1:
                    nc.sync.dma_start(
                        out=outp[sq0 + 128 * m:sq0 + 128 * (m + 1), :],
                        in_=o_sb[:])

            # ---- emission schedule ----
            # preamble: only the projection units the first attention
            # window needs up front (q cc0, k cc0 of slice 0); everything
            # else streams in as chunk-loop fills so the EXP pipeline
            # starts ~15 us earlier.  DMA issue order tracks first use.
            yt0 = load_slice(yT, 0)
            load_wk()
            xt0 = load_slice(xT, 0)
            load_wv()
            proj_q_cc(0, 0, yt0, pvb, "pvB")
            proj_k_cc(0, 0, xt0, pvb, "pvB")
            load_tail_params()

            xts = {0: xt0}
            yts = {0: yt0}

            def kv_batch(sb):
                """k (both cc) + first v part of slice sb, with its DMA."""
                xts[sb] = load_slice(xT, sb)
                proj_k_cc(sb, 0, xts[sb], pvb, "pvB")
                proj_k_cc(sb, 1, xts[sb], pvb, "pvB")
                proj_v_sc4(sb, 0, xts[sb], pvb, "pvB")

            def q_batch(sb):
                yts[sb] = load_slice(yT, sb)
                proj_q_cc(sb, 0, yts[sb], pva, "pvA")
                proj_q_cc(sb, 1, yts[sb], pva, "pvA")

            prev_pv = None     # (blk, pair, tiles) pending epilogue
            next_ats = None    # next window's chunk-0/1 at tiles
            for blk in range(NBLK):
                for pair in range(2):
                    pool, tag = (pva, "pvA") if pair == 0 else (pvb, "pvB")
                    fill_pool, fill_tag = ((pvb, "pvB") if pair == 0
                                           else (pva, "pvA"))
                    pv_ps = [pool.tile([128, SQB], F32, tag=tag,
                                       name=f"pv{hh}")[:HD + 1, :]
                             for hh in range(2)]
                    if blk == 0 and pair == 0:
                        # first v parts BEFORE the first scores: the PE
                        # semaphore increments are batched, so anything
                        # emitted between the scores and their EXP
                        # consumers stretches the EXP's wait threshold
                        proj_v_sc4(0, 0, xt0, pvb, "pvB")
                        proj_v_sc4(0, 1, xt0, pvb, "pvB")
                    # chunk-0/1 scores: prefetched at the tail of the
                    # previous window (so the EXP stream has no boundary
                    # bubble); the first window emits its own
                    if next_ats is None:
                        ats = [attn_scores(blk, pair, 0),
                               attn_scores(blk, pair, 1)]
                    else:
                        ats = next_ats
                    next_ats = None
                    pending_epi_b = None
                    if prev_pv is not None:
                        pblk, ppair, ptiles = prev_pv
                        epilogue_a(pblk, ppair, ptiles)
                        pending_epi_b = (pblk, ppair)
                    attn_pv(pair, 0, ats[0], pv_ps)
                    attn_pv(pair, 1, ats[1], pv_ps)
                    if blk == 0 and pair == 0:
                        # pv(2) consumes this at the first loop iteration
                        proj_v_sc4(0, 2, xt0, pvb, "pvB")
                    # interleaved fill work for the PE in this window;
                    # {fire_after_chunk: [emissions]}.  kT/v slice sb MUST
                    # be emitted before chunk 4*sb reads it.  outproj of
                    # blk-1 is split 4+4 units across blk's pair0 (units
                    # 0-3, after this window's epilogue_b at sc 4/6 has
                    # normalized blk-1/pair1) and pair1 (units 4-7) so
                    # each window carries ~3.5 us of outproj instead of 7.
                    if blk == 0 and pair == 0:
                        # fills(sc) emit AFTER pv(sc): v_sb[k] must be
                        # emitted at fills(k-1) or earlier; kT slice sb
                        # before scores(4*sb) i.e. fills(4*sb-1).
                        fills = {
                            2: [lambda: proj_v_sc4(0, 3, xt0, pvb, "pvB")],
                            3: [lambda: kv_batch(1),
                                lambda: proj_v_sc4(1, 1, xts[1], pvb,
                                                   "pvB")],
                            5: [lambda: proj_v_sc4(1, 2, xts[1], pvb,
                                                   "pvB"),
                                lambda: proj_v_sc4(1, 3, xts[1], pvb,
                                                   "pvB")],
                            6: [lambda: proj_k_cc(0, 1, xt0, pvb, "pvB"),
                                lambda: proj_q_cc(0, 1, yt0, pvb, "pvB")],
                            7: [lambda: kv_batch(2),
                                lambda: proj_v_sc4(2, 1, xts[2], pvb,
                                                   "pvB")],
                            9: [lambda: proj_v_sc4(2, 2, xts[2], pvb,
                                                   "pvB")],
                            10: [lambda: proj_v_sc4(2, 3, xts[2], pvb,
                                                    "pvB")],
                            11: [lambda: kv_batch(3),
                                 lambda: proj_v_sc4(3, 1, xts[3], pvb,
                                                    "pvB")],
                            13: [lambda: proj_v_sc4(3, 2, xts[3], pvb,
                                                    "pvB")],
                            14: [lambda: proj_v_sc4(3, 3, xts[3], pvb,
                                                    "pvB")],
                        }
                    elif blk == 0 and pair == 1:
                        fills = {3: [lambda: q_batch(1)],
                                 7: [lambda: q_batch(2)],
                                 11: [lambda: q_batch(3)]}
                    else:
                        # 5 units in pair0 windows (they run ~1us under
                        # the ACT envelope), 3 in the outproj-heavier
                        # pair1 windows
                        lo, n = (0, 5) if pair == 0 else (5, 3)
                        base = 7 if pair == 0 else 2
                        fills = {base + s: [lambda u=u: outproj_unit(
                            blk - 1, u // 2, u % 2, fill_pool, fill_tag)]
                            for s, u in enumerate(range(lo, lo + n))}
                    is_last = (blk == NBLK - 1 and pair == 1)
                    nblk, npair = (blk, 1) if pair == 0 else (blk + 1, 0)
                    for sc in range(2, NKC):
                        at = attn_scores(blk, pair, sc)
                        if sc == NKC - 1 and not is_last:
                            # prefetch the next window's chunk-0/1 scores
                            # BETWEEN this window's last scores and its
                            # last PVs: in the strict PE FIFO they then
                            # run as soon as the psc slots free (EXP 14
                            # and 15), so the EXP stream crosses the
                            # window boundary with no bubble.
                            next_ats = [attn_scores(nblk, npair, 0),
                                        attn_scores(nblk, npair, 1)]
                        attn_pv(pair, sc, at, pv_ps)
                        if pending_epi_b is not None and sc in (4, 6):
                            epilogue_b(*pending_epi_b, hh=(0 if sc == 4
                                                           else 1))
                        for f in fills.get(sc, ()):
                            f()
                    prev_pv = (blk, pair, pv_ps)

            # tail: last pair epilogue (d-chain first -- it gates the
            # final outproj).  The first 4 units' pair0 matmuls depend
            # only on the long-normalized nv[pair0], so they stream on
            # the PE while the DVE runs the d-chain; their pair1 halves
            # and the remaining units follow the normalize muls.
            pblk, ppair, ptiles = prev_pv
            epilogue_a(pblk, ppair, ptiles, tail=True)
            sq0 = SQB * (NBLK - 1)
            tail_ps = []
            tail_osb = []
            for u in range(4):
                pool, tag = (pva, "pvA") if u % 2 == 0 else (pvb, "pvB")
                m, dcb = u // 2, u % 2
                if dcb == 0:
                    osb_box[0] = stream.tile([128, D], FP16, tag="o_sb",
                                             name="o_sb")
                tail_osb.append(osb_box[0])
                o_ps = pool.tile([128, SQB], F32, tag=tag, name="o_ps")
                nc.tensor.matmul(
                    o_ps[:],
                    nv_sb[0][:, sq0 + 128 * m:sq0 + 128 * (m + 1)],
                    wo_sb[:, 512 * dcb:512 * (dcb + 1)],
                    start=True, stop=False)
                tail_ps.append(o_ps)
            epilogue_b(pblk, ppair, hh=0)
            epilogue_b(pblk, ppair, hh=1)
            for u in range(4):
                m, dcb = u // 2, u % 2
                o_ps, o_sb = tail_ps[u], tail_osb[u]
                nc.tensor.matmul(
                    o_ps[:],
                    nv_sb[1][:, sq0 + 128 * m:sq0 + 128 * (m + 1)],
                    wo_sb[:, D + 512 * dcb:D + 512 * (dcb + 1)],
                    start=False, stop=True)
                nc.vector.tensor_add(o_sb[:, 512 * dcb:512 * (dcb + 1)],
                                     o_ps[:], bo_bc[:, 512 * dcb:
                                                    512 * (dcb + 1)])
                nc.sync.dma_start(
                    out=outp[sq0 + 128 * m:sq0 + 128 * (m + 1),
                             512 * dcb:512 * (dcb + 1)],
                    in_=o_sb[:, 512 * dcb:512 * (dcb + 1)])
            for u in range(4, 2 * (SQB // 128)):
                pool, tag = (pva, "pvA") if u % 2 == 0 else (pvb, "pvB")
                outproj_unit(NBLK - 1, u // 2, u % 2, pool, tag,
                             dma_halves=True)

    nc.compile()
    return nc


last_results = None


def kernel(x, y, mask, Wkv, bkv, Wq, bq, Wo, bo):
    x = np.asarray(x, dtype=np.float32)
    y = np.asarray(y, dtype=np.float32)
    Wkv = np.asarray(Wkv, dtype=np.float32)
    bkv = np.asarray(bkv, dtype=np.float32)
    Wq = np.asarray(Wq, dtype=np.float32)
    bq = np.asarray(bq, dtype=np.float32)
    Wo = np.asarray(Wo, dtype=np.float32)
    bo = np.asarray(bo, dtype=np.float32)

    wkv3 = Wkv.reshape(D, H, 2 * HD)
    bv = bkv.reshape(H, 2 * HD)[:, HD:].reshape(H * HD)
    # v-bias folded into the output bias; each of the 4 partial sums per
    # batch carries bo_eff/4 so the host-side reduce reproduces bo_eff.
    bo_eff4 = ((bv @ Wo + bo) / GROUP).astype(np.float32)

    def chunk_major(a):
        """[D, M] -> [128, NDC*M]: SBUF layout, contraction-chunk-major."""
        d, m = a.shape
        return np.ascontiguousarray(
            a.reshape(d // 128, 128, m).transpose(1, 0, 2).reshape(
                128, (d // 128) * m))

    def slice_major(a):
        """[D, S] -> [128, NBLK*NDC*SKB]: per 512-col slice, chunk-major
        contiguous."""
        d, s = a.shape
        return np.ascontiguousarray(
            a.reshape(d // 128, 128, s // SKB, SKB)
            .transpose(1, 2, 0, 3).reshape(128, s * (d // 128)))

    nc = build_kernel()
    in_maps = []
    for c in range(N_CORES):
        b, j = divmod(c, GROUP)
        hs = HPC * j
        f16 = np.float16
        in_maps.append({
            "yT": slice_major(y[b].T).astype(f16),
            "xT": slice_major(x[b].T).astype(f16),
            "wq": chunk_major(Wq[:, HD * hs:HD * (hs + HPC)]).astype(f16),
            "wk": chunk_major(
                wkv3[:, hs:hs + HPC, :HD].reshape(D, NV)).astype(f16),
            "wv": chunk_major(
                wkv3[:, hs:hs + HPC, HD:].reshape(D, NV)).astype(f16),
            "wo": chunk_major(
                Wo[HD * hs:HD * (hs + HPC), :]).astype(f16),
            "bq": np.ascontiguousarray(
                bq[HD * hs:HD * (hs + HPC)].reshape(2, 128).T),
            "bo": bo_eff4,
        })

    import os
    trace = bool(os.environ.get("KERNEL_TRACE"))
    res = run_bass_kernel_spmd(nc, in_maps, core_ids=list(range(N_CORES)),
                               trace=trace)
    global last_results
    last_results = res

    full = np.empty((B, S, D), dtype=np.float32)
    for b in range(B):
        acc = res.results[GROUP * b]["outp"].astype(np.float32)
        for j in range(1, GROUP):
            acc = acc + res.results[GROUP * b + j]["outp"].astype(np.float32)
        full[b] = acc
    return full
